# revision 36
# baseline (speedup 1.0000x reference)
"""NeRF render kernel for 8 Trainium2 NeuronCores.

Data-parallel over rays: core k handles rays [2048*k, 2048*(k+1)).
Per core: positional encoding + 3-layer MLP (39->256->256->4) over
131072 points in feature-major layout (features on partitions, points
on the free dim), then alpha compositing via triangular-matrix matmul
cumulative sums.

Point permutation inside a core: partitions are q = rp*64 + s (rp =
ray parity, s = sample), columns J = 128*g + i denote the ray pair
(16i + 2g, 16i + 2g + 1).  The host pre-transposes t_rand into this
[q, J] layout and folds origins/directions/z-offsets into per-point
affine coefficients A, B (in units of turns: x/(2*pi)) so the device
computes pts_turns = A + B*t directly.  Trig arguments are range-
reduced mod 1 in turns (exact float32 round-trick arithmetic) with
per-level doubling; the Sin activation applies scale=2*pi for free.

Feature rows (f-major sf layout, col = f*64 + j): 0-17 sin(2^l x_c),
18-35 cos, 36-38 raw x_c, 39 constant 1 (bias row folding b0 into W0).
Layer 1 runs as two fp8e4 DoubleRow matmuls (k-subtile packed), with
an fp16 fallback behind USE_FP8_L1.
"""

import sys
import numpy as np

sys.path.insert(0, "/opt/trn_rl_repo")

S = 64
L = 6
NCORES = 8
B = 16384
BC = B // NCORES          # rays per core
NP = BC * S               # points per core
NBLK = NP // 128          # 1024 ray-pair blocks (J)
NGRP = 8                  # groups of 128 blocks
NEAR, FAR = 2.0, 6.0
DELTA = (FAR - NEAR) / S
TWO_PI = float(2.0 * np.pi)
INV2PI = float(np.float32(1.0 / (2.0 * np.pi)))
MAGIC = 12582912.0  # 1.5 * 2**23: float32 round-to-int trick

USE_FP8_L1 = False

_CACHE = {}
PROFILE = False  # test harness sets True to collect an NTFF trace


def _split_waits(nc, mybir):
    """TRN2 allows one sem wait per instruction (two for EventSemaphore);
    this walrus build rejects over-limit instructions, so move excess waits
    onto chained NOPs on the same engine just before the instruction."""
    ctr = 0
    for fn in nc.m.functions:
        for bb in fn.blocks:
            changed = False
            out = []
            for inst in bb.instructions:
                si = inst.sync_info
                cap = 2 if isinstance(inst, mybir.InstEventSemaphore) else 1
                if si is not None and si.on_wait and len(si.on_wait) > cap:
                    waits = list(si.on_wait)
                    for w in waits[:-cap]:
                        nop = mybir.InstNoOp(
                            name=f"wsplit-{ctr}", ins=[], outs=[]
                        )
                        ctr += 1
                        nop.engine = inst.engine
                        nop.sync_info = mybir.SyncInfo(on_wait=[w], on_update=[])
                        nc.register_instruction(nop)
                        out.append(nop)
                    si.on_wait = waits[-cap:]
                    changed = True
                out.append(inst)
            if changed:
                bb.instructions = out
    return ctr


def _build():
    import concourse.bass as bass
    import concourse.mybir as mybir
    import concourse.tile as tile

    dt = mybir.dt
    AF = mybir.ActivationFunctionType
    OP = mybir.AluOpType
    F32 = dt.float32
    F32R = dt.float32r
    F16 = dt.float16
    F8 = dt.float8e4

    nc = bass.Bass()

    # ---- DRAM I/O ----
    # per-group affine coefs in turns: pts_t = A + B * t
    a_d = nc.dram_tensor("acf", [NGRP, 128, 384], F32, kind="ExternalInput")
    b_d = nc.dram_tensor("bcf", [NGRP, 128, 384], F32, kind="ExternalInput")
    tq_d = nc.dram_tensor("tq", [NGRP, 128, 128], F32, kind="ExternalInput")
    w0_d = nc.dram_tensor("w0p", [40, 256], F16, kind="ExternalInput")
    if USE_FP8_L1:
        w1_d = nc.dram_tensor("w1dr", [128, 512], F8, kind="ExternalInput")
    else:
        w1_d = nc.dram_tensor("w1", [256, 256], F16, kind="ExternalInput")
    w2_d = nc.dram_tensor("w2h", [128, 8], F16, kind="ExternalInput")
    b1_d = nc.dram_tensor("b1t", [128, 2], F32, kind="ExternalInput")
    b2_d = nc.dram_tensor("b2t", [128, 4], F32, kind="ExternalInput")
    ltri_d = nc.dram_tensor("ltri", [128, 256], F32, kind="ExternalInput")
    sel2_d = nc.dram_tensor("sel2", [128, 2], F32R, kind="ExternalInput")
    identh_d = nc.dram_tensor("identh", [128, 128], F16, kind="ExternalInput")
    out_d = nc.dram_tensor("out", [NGRP, 2, 384], F32, kind="ExternalOutput")

    HB = 64  # blocks per half
    TPH = HB // 4  # 512-point tiles per half

    with tile.TileContext(nc) as tc:
        with (
            tc.tile_pool(name="consts", bufs=1) as cpool,
            tc.tile_pool(name="o2", bufs=NGRP) as o2pool,
        ):
            # ---- constants / weights (small, upfront) ----
            # w0 at partitions 0-39 and a replica at 64-103 (L0 row-packing)
            w0s = cpool.tile([128, 256], F16, tag="w0s")
            nc.sync.dma_start(w0s[0:40, :], w0_d[:])
            nc.sync.dma_start(w0s[64:104, :], w0_d[:])
            if USE_FP8_L1:
                w1dr = cpool.tile([128, 512], F8, tag="w1dr")
                nc.sync.dma_start(w1dr[:], w1_d[:])
                w1v = w1dr.rearrange("p (k m) -> p k m", k=2)
            else:
                w1s0 = cpool.tile([128, 256], F16, tag="w1s0")
                nc.sync.dma_start(w1s0[:], w1_d[0:128, :])
                w1s1 = cpool.tile([128, 256], F16, tag="w1s1")
                nc.sync.dma_start(w1s1[:], w1_d[128:256, :])
            w2s = cpool.tile([128, 8], F16, tag="w2s")
            nc.sync.dma_start(w2s[:], w2_d[:])
            b1t = cpool.tile([128, 2], F32, tag="b1t")
            nc.sync.dma_start(b1t[:], b1_d[:])
            b2t = cpool.tile([128, 4], F32, tag="b2t")
            nc.sync.dma_start(b2t[:], b2_d[:])
            ltri = cpool.tile([128, 256], F32, tag="ltri")
            nc.sync.dma_start(ltri[:], ltri_d[:])
            sel2 = cpool.tile([128, 2], F32R, tag="sel2")
            nc.sync.dma_start(sel2[:], sel2_d[:])
            identh = cpool.tile([128, 128], F16, tag="identh")
            nc.sync.dma_start(identh[:], identh_d[:])
            zerot = cpool.tile([128, 1], F32, tag="zerot")
            nc.vector.memset(zerot[:], 0.0)

            with (
                tc.tile_pool(name="gin", bufs=2) as ginpool,
                tc.tile_pool(name="gpt", bufs=2) as gptpool,
                tc.tile_pool(name="sa", bufs=2) as sapool,
                tc.tile_pool(name="kb", bufs=2) as kbpool,
                tc.tile_pool(name="sf", bufs=2) as sfpool,
                tc.tile_pool(name="fs", bufs=4) as fspool,
                tc.tile_pool(name="h0s", bufs=2) as h0spool,
                tc.tile_pool(name="h1s", bufs=4) as h1spool,
                tc.tile_pool(name="tpP", bufs=1, space="PSUM") as tp_pool,
                tc.tile_pool(name="h0P", bufs=2, space="PSUM") as h0_pool,
                tc.tile_pool(name="h1P", bufs=2, space="PSUM") as h1_pool,
                tc.tile_pool(name="oP", bufs=1, space="PSUM") as o_pool,
            ):
                o2s = []
                for g in range(NGRP):
                    # ---- phase A (per group): pts in turns + range reduce
                    # First groups' loads go on the second (Activation) DGE
                    # ring so the startup DMA burst runs 2-wide.
                    dge = nc.scalar if g < 2 else nc.sync
                    at = ginpool.tile([128, 384], F32, tag="at")
                    dge.dma_start(at[:], a_d[g])
                    bt = ginpool.tile([128, 384], F32, tag="bt")
                    dge.dma_start(bt[:], b_d[g])
                    tqg = ginpool.tile([128, 128], F32, tag="tqg")
                    dge.dma_start(tqg[:], tq_d[g])

                    ptst = gptpool.tile([128, 384], F32, tag="ptst")
                    ptsr = gptpool.tile([128, 384], F32, tag="ptsr")
                    praw = gptpool.tile([128, 384], F32, tag="praw")
                    tqb = tqg.unsqueeze(1).broadcast_to([128, 3, 128])
                    p3 = ptst.rearrange("p (c j) -> p c j", c=3)
                    nc.vector.tensor_tensor(p3, bt.rearrange("p (c j) -> p c j", c=3), tqb, op=OP.mult)
                    nc.vector.tensor_tensor(ptst[:], ptst[:], at[:], op=OP.add)
                    # raw radians for features 36-38
                    nc.vector.tensor_scalar(praw[:], ptst[:], TWO_PI, None, op0=OP.mult)
                    # r0 = ptst - round(ptst)  (exact in fp32)
                    kg = gptpool.tile([128, 384], F32, tag="kg")
                    nc.vector.tensor_scalar(kg[:], ptst[:], MAGIC, None, op0=OP.add)
                    nc.vector.tensor_scalar(kg[:], kg[:], MAGIC, None, op0=OP.subtract)
                    nc.vector.tensor_tensor(ptsr[:], ptst[:], kg[:], op=OP.subtract)

                    og = o_pool.tile([128, 512], F32, tag="og")

                    # Software pipeline: L1 for tile t is emitted during
                    # tile t+1 and L2 during t+2, so the in-order PE queue
                    # never waits on same-tile activation drains.
                    pend_l1 = []
                    pend_l2 = []

                    def emit_l2(h1pair, jbase):
                        for jp in range(4):
                            jj = jbase + jp
                            nc.tensor.matmul(
                                og[:, 4 * jj : 4 * (jj + 1)],
                                h1pair[1][:, 128 * jp : 128 * (jp + 1)],
                                w2s[:, 4:8],
                                start=True,
                                stop=False,
                            )
                            nc.tensor.matmul(
                                og[:, 4 * jj : 4 * (jj + 1)],
                                h1pair[0][:, 128 * jp : 128 * (jp + 1)],
                                w2s[:, 0:4],
                                start=False,
                                stop=True,
                            )

                    def emit_l1(h0s, jbase):
                        # half 1 first so its vector drain overlaps half 0's
                        # matmuls; L2 consumes half 1 (start) before half 0.
                        h1ss = [None, None]
                        for h in (1, 0):
                            h1p = h1_pool.tile([128, 512], F32, tag="h1p")
                            nc.tensor.matmul(
                                h1p[:],
                                w1s0[:, 128 * h : 128 * (h + 1)],
                                h0s[:, 0:512],
                                start=True,
                                stop=False,
                            )
                            nc.tensor.matmul(
                                h1p[:],
                                w1s1[:, 128 * h : 128 * (h + 1)],
                                h0s[:, 512:1024],
                                start=False,
                                stop=True,
                            )
                            h1s = h1spool.tile([128, 512], F16, tag="h1s")
                            with tc.high_priority():
                                if h == 1:
                                    nc.vector.tensor_scalar(
                                        h1s[:], h1p[:], b1t[:, 1:2], 0.0,
                                        op0=OP.add, op1=OP.max,
                                    )
                                else:
                                    nc.scalar.activation(
                                        h1s[:], h1p[:], AF.Relu,
                                        bias=b1t[:, 0:1],
                                    )
                            h1ss[h] = h1s
                        pend_l2.append((h1ss, jbase))

                    for half in range(2):
                        # ---- per-half: doubling chain (turns) + sin ----
                        sa = sapool.tile([128, 2304], F32, tag="sa")
                        sav = sa.rearrange("p (f j) -> p f j", j=HB)
                        # r0 half-view [128, 3, 64]
                        r0v = ptsr.rearrange("p (c j) -> p c j", c=3)[
                            :, :, HB * half : HB * (half + 1)
                        ]
                        prv = praw.rearrange("p (c j) -> p c j", c=3)[
                            :, :, HB * half : HB * (half + 1)
                        ]
                        # sin chunk l=0
                        nc.vector.tensor_copy(sav[:, 0:3, :], r0v)
                        # sin l=1..5: r_l = 2 r_{l-1} - round(2 r_{l-1})
                        for l in range(1, L):
                            prev = sa[:, 192 * (l - 1) : 192 * l]
                            cur = sa[:, 192 * l : 192 * (l + 1)]
                            kb = kbpool.tile([128, 192], F32, tag="kb")
                            nc.vector.tensor_scalar(
                                kb[:], prev, 2.0, MAGIC, op0=OP.mult, op1=OP.add
                            )
                            nc.vector.tensor_scalar(
                                kb[:], kb[:], MAGIC, None, op0=OP.subtract
                            )
                            nc.vector.scalar_tensor_tensor(
                                cur, prev, 2.0, kb[:], op0=OP.mult, op1=OP.subtract
                            )
                        # cos l: c_l = r_l + 0.25 - [r_l > 0.25]
                        for l in range(L):
                            rl = sa[:, 192 * l : 192 * (l + 1)]
                            cl = sa[:, 1152 + 192 * l : 1152 + 192 * (l + 1)]
                            kb = kbpool.tile([128, 192], F32, tag="kb")
                            nc.vector.tensor_scalar(
                                kb[:], rl, 0.25, 1.0, op0=OP.is_gt, op1=OP.mult
                            )
                            nc.vector.scalar_tensor_tensor(
                                cl, rl, 0.25, kb[:], op0=OP.add, op1=OP.subtract
                            )
                        # sf (f-major): 0:2304 trig, 2304:2496 raw, 2496:2560 ones
                        sf = sfpool.tile([128, 2560], F16, tag="sf")
                        nc.scalar.activation(
                            sf[:, 0:2304], sa[:], AF.Sin, scale=TWO_PI
                        )
                        nc.vector.tensor_copy(
                            sf.rearrange("p (f j) -> p f j", j=HB)[:, 36:39, :], prv
                        )
                        nc.vector.memset(sf[:, 2496:2560], 1.0)

                        sfv = sf.rearrange("p (f j) -> p f j", j=HB)
                        for itl in range(TPH):
                            # PE-transpose 4 blocks -> feature-major fp16.
                            # Two 1KB halves of one PSUM bank, manual ping-pong.
                            tpfull = tp_pool.tile([40, 1024], F16, tag="tp")
                            tp = tpfull[:, 512 * (itl % 2) : 512 * (itl % 2) + 512]
                            for jp in range(4):
                                jj = 4 * itl + jp
                                nc.tensor.transpose(
                                    tp[:, 128 * jp : 128 * (jp + 1)],
                                    sfv[:, :, jj],
                                    identh[:],
                                )
                            fs = fspool.tile([128, 512], F16, tag="fs")
                            with tc.high_priority():
                                nc.vector.tensor_copy(fs[0:40, :], tp[:])
                            # duplicate features at partitions 64-103 so L0
                            # can run as two concurrent row-tiles
                            nc.sync.dma_start(fs[64:104, :], fs[0:40, :])
                            # L0 (bias folded via ones row): one concurrent
                            # row-tile pair — rows 0-39 compute hidden half 0,
                            # rows 64-103 (duplicated features) compute half 1.
                            # Outputs land in different PSUM banks.
                            h0p = h0_pool.tile([128, 1024], F32, tag="h0p")
                            nc.tensor.matmul(
                                h0p[:, 0:512], w0s[0:40, 0:128], fs[0:40, :]
                            )
                            nc.tensor.matmul(
                                h0p[:, 512:1024], w0s[64:104, 128:256],
                                fs[64:104, :],
                            )
                            h0s = h0spool.tile([128, 1024], F16, tag="h0s")
                            with tc.high_priority():
                                nc.scalar.activation(h0s[:], h0p[:], AF.Relu)
                            # previous tile's L2 goes ahead of this tile's L1
                            # so ready PE work can fill the h0-drain window
                            if len(pend_l2) > 0:
                                emit_l2(*pend_l2.pop(0))
                            emit_l1(h0s, 64 * half + 4 * itl)
                    while pend_l2:
                        emit_l2(*pend_l2.pop(0))
                    # ---- drain O psum -> O2 sbuf (bias, relu on sigma) ----
                    o2 = o2pool.tile([128, 512], F32, tag="o2")
                    orgb = og.rearrange("p (j c) -> p j c", c=4)[:, :, 0:3]
                    o2rgb = o2.rearrange("p (j c) -> p j c", c=4)[:, :, 0:3]
                    brgb = b2t[:, 0:3].unsqueeze(1).broadcast_to([128, 128, 3])
                    nc.vector.tensor_tensor(o2rgb, orgb, brgb, op=OP.add)
                    osig = og.rearrange("p (j c) -> p j c", c=4)[:, :, 3]
                    o2sig = o2.rearrange("p (j c) -> p j c", c=4)[:, :, 3]
                    zbc = zerot[:, 0:1].broadcast_to([128, 128])
                    nc.vector.scalar_tensor_tensor(
                        o2sig, osig, b2t[:, 3:4], zbc, op0=OP.add, op1=OP.max
                    )
                    o2s.append(o2)

            # ---- phase C: compositing (tanh-sigmoid + exp scans) ----
            tc.no_sync_barrier()
            with (
                tc.tile_pool(name="cS", bufs=2) as cspool,
                tc.tile_pool(name="cP", bufs=2, space="PSUM") as cppool,
            ):
                for g in range(NGRP):
                    o2 = o2s[g]
                    o2v = o2.rearrange("p (j c) -> p j c", c=4)
                    # sigmoid(x) = 0.5*tanh(x/2) + 0.5 (0.5s folded into sel2)
                    e = cspool.tile([128, 384], F32, tag="e")
                    nc.scalar.activation(
                        e.rearrange("p (j c) -> p j c", c=3),
                        o2v[:, :, 0:3],
                        AF.Tanh,
                        scale=0.5,
                    )
                    # scans: exclusive & inclusive cumsum of sigma over s
                    scp = cppool.tile([128, 256], F32, tag="scp")
                    sig = o2v[:, :, 3]
                    nc.tensor.matmul(scp[:, 0:128], ltri[:, 0:128], sig)
                    nc.tensor.matmul(scp[:, 128:256], ltri[:, 128:256], sig)
                    texin = cspool.tile([128, 256], F32, tag="texin")
                    nc.scalar.activation(texin[:], scp[:], AF.Exp, scale=-DELTA)
                    wt = cspool.tile([128, 128], F32, tag="wt")
                    nc.vector.tensor_tensor(
                        wt[:], texin[:, 0:128], texin[:, 128:256], op=OP.subtract
                    )
                    # wr = (tanh + 1) * w   (the 0.5 lives in sel2)
                    wr = cspool.tile([128, 384], F32R, tag="wr")
                    nc.vector.scalar_tensor_tensor(
                        wr.rearrange("p (j c) -> p j c", c=3),
                        e.rearrange("p (j c) -> p j c", c=3),
                        1.0,
                        wt.unsqueeze(2).broadcast_to([128, 128, 3]),
                        op0=OP.add,
                        op1=OP.mult,
                    )
                    rp_ = cppool.tile([2, 384], F32, tag="rp")
                    nc.tensor.matmul(rp_[:], sel2[:], wr[:])
                    outs = cspool.tile([2, 384], F32, tag="outs")
                    nc.vector.tensor_copy(outs[:], rp_[:])
                    nc.sync.dma_start(out_d[g], outs[:])

    _split_waits(nc, mybir)
    return nc


def _host_prep(origins, directions, t_rand, W0, b0, W1, b1, W2, b2):
    """Build per-core input maps (all numpy, cheap)."""
    import ml_dtypes

    f32 = np.float32
    # F-row order: rows 3l+c = sin freq l coord c; 18+3l+c = cos; 36..38 pts;
    # row 39 = bias (ones feature).
    perm = np.zeros(39, np.int64)
    perm[36:39] = (0, 1, 2)
    for l in range(L):
        for c in range(3):
            perm[3 * l + c] = 3 + 6 * l + c
            perm[18 + 3 * l + c] = 3 + 6 * l + 3 + c
    w0p = np.empty((40, 256), np.float16)
    w0p[0:39] = W0[perm].astype(np.float16)
    w0p[39] = b0.astype(np.float16)

    if USE_FP8_L1:
        # [p, k*256 + m] = W1[k*128 + p, m]
        w1dr = np.empty((128, 512), ml_dtypes.float8_e4m3)
        w1dr[:, 0:256] = W1[0:128, :].astype(ml_dtypes.float8_e4m3)
        w1dr[:, 256:512] = W1[128:256, :].astype(ml_dtypes.float8_e4m3)
        w1_payload = ("w1dr", w1dr)
    else:
        w1_payload = ("w1", W1.astype(np.float16))

    w2h = np.empty((128, 8), np.float16)
    w2h[:, 0:4] = W2[0:128].astype(np.float16)
    w2h[:, 4:8] = W2[128:256].astype(np.float16)
    b1t = np.ascontiguousarray(b1.reshape(2, 128).T).astype(f32)
    b2t = np.broadcast_to(b2.astype(f32), (128, 4)).copy()

    q = np.arange(128)
    rp = q // 64
    s = q % 64
    zc = (NEAR + DELTA * s).astype(np.float64)  # [128] per-partition z offset

    # ltri: cols 0..127 exclusive, 128..255 inclusive
    kk = q
    krp = kk // 64
    kj = kk % 64
    same = (krp[:, None] == rp[None, :])
    ltri = np.zeros((128, 256), f32)
    ltri[:, 0:128] = (same & (kj[:, None] < s[None, :])).astype(f32)
    ltri[:, 128:256] = (same & (kj[:, None] <= s[None, :])).astype(f32)
    # 0.5 from the tanh-sigmoid identity folded in here
    sel2 = (0.5 * (krp[:, None] == np.arange(2)[None, :])).astype(f32)
    identh = np.eye(128, dtype=np.float16)

    # ray_of[J, rp] = 16*(J%128) + 2*(J//128) + rp
    J = np.arange(NBLK)
    ray_of = (16 * (J % 128))[:, None] + (2 * (J // 128))[:, None] + np.arange(2)[None, :]

    in_maps = []
    for core in range(NCORES):
        o = origins[core * BC : (core + 1) * BC].astype(np.float64)
        d = directions[core * BC : (core + 1) * BC].astype(np.float64)
        t = t_rand[core * BC : (core + 1) * BC].astype(f32)
        rays_qJ = ray_of[:, :].T[rp]  # [128, NBLK]: ray_of[J, rp(q)]
        # tq[q, J] = t[ray, s(q)]
        tq = t[rays_qJ, s[:, None]]  # [128, NBLK]
        tq = np.ascontiguousarray(tq.reshape(128, NGRP, 128).transpose(1, 0, 2))
        # affine coefs in turns: pts_t[c] = A + B * t
        # x_c = o_c + d_c * (zc(q) + DELTA * t)
        oe = o[rays_qJ]  # [128, NBLK, 3]
        de = d[rays_qJ]
        A = (oe + de * zc[:, None, None]) * INV2PI   # [128, NBLK, 3]
        Bc = de * (DELTA * INV2PI)
        # -> [NGRP, 128, 384] with col = c*128 + j
        A = A.reshape(128, NGRP, 128, 3).transpose(1, 0, 3, 2).reshape(NGRP, 128, 384)
        Bc = Bc.reshape(128, NGRP, 128, 3).transpose(1, 0, 3, 2).reshape(NGRP, 128, 384)
        in_map = {
            "acf": np.ascontiguousarray(A).astype(f32),
            "bcf": np.ascontiguousarray(Bc).astype(f32),
            "tq": tq,
            "w0p": w0p,
            w1_payload[0]: w1_payload[1],
            "w2h": w2h,
            "b1t": b1t,
            "b2t": b2t,
            "ltri": ltri,
            "sel2": sel2,
            "identh": identh,
        }
        in_maps.append(in_map)
    return in_maps


def kernel(origins, directions, t_rand, W0, b0, W1, b1, W2, b2, near, far,
           **kw):
    assert int(near) == 2 and int(far) == 6
    from concourse.bass_utils import run_bass_kernel_spmd

    if "nc" not in _CACHE:
        _CACHE["nc"] = _build()
    nc = _CACHE["nc"]

    in_maps = _host_prep(
        np.asarray(origins), np.asarray(directions), np.asarray(t_rand),
        np.asarray(W0), np.asarray(b0), np.asarray(W1), np.asarray(b1),
        np.asarray(W2), np.asarray(b2),
    )
    res = run_bass_kernel_spmd(
        nc, in_maps, core_ids=list(range(NCORES)), trace=PROFILE
    )
    _CACHE["last_results"] = res
    out = np.empty((B, 3), np.float32)
    for core in range(NCORES):
        oc = res.results[core]["out"].reshape(NGRP, 2, 128, 3)
        # group g holds blocks J = 128*g + i ; ray = 16*i + 2*g + rp
        for g in range(NGRP):
            for rpp in range(2):
                rays = core * BC + 16 * np.arange(128) + 2 * g + rpp
                out[rays] = oc[g, rpp]
    return out


# revision 37
# speedup vs baseline: 1.0327x; 1.0327x over previous
"""NeRF render kernel for 8 Trainium2 NeuronCores.

Data-parallel over rays: core k handles rays [2048*k, 2048*(k+1)).
Per core: positional encoding + 3-layer MLP (39->256->256->4) over
131072 points in feature-major layout (features on partitions, points
on the free dim), then alpha compositing via triangular-matrix matmul
cumulative sums.

Point permutation inside a core: partitions are q = rp*64 + s (rp =
ray parity, s = sample), columns J = 128*g + i denote the ray pair
(16i + 2g, 16i + 2g + 1).  The host pre-transposes t_rand into this
[q, J] layout and folds origins/directions/z-offsets into per-point
affine coefficients A, B (in units of turns: x/(2*pi)) so the device
computes pts_turns = A + B*t directly.  Trig arguments are range-
reduced mod 1 in turns (exact float32 round-trick arithmetic) with
per-level doubling; the Sin activation applies scale=2*pi for free.

Feature rows (f-major sf layout, col = f*64 + j): 0-17 sin(2^l x_c),
18-35 cos, 36-38 raw x_c, 39 constant 1 (bias row folding b0 into W0).
Layer 1 runs as two fp8e4 DoubleRow matmuls (k-subtile packed), with
an fp16 fallback behind USE_FP8_L1.
"""

import sys
import numpy as np

sys.path.insert(0, "/opt/trn_rl_repo")

S = 64
L = 6
NCORES = 8
B = 16384
BC = B // NCORES          # rays per core
NP = BC * S               # points per core
NBLK = NP // 128          # 1024 ray-pair blocks (J)
NGRP = 8                  # groups of 128 blocks
NEAR, FAR = 2.0, 6.0
DELTA = (FAR - NEAR) / S
TWO_PI = float(2.0 * np.pi)
INV2PI = float(np.float32(1.0 / (2.0 * np.pi)))
MAGIC = 12582912.0  # 1.5 * 2**23: float32 round-to-int trick

USE_FP8_L1 = False

_CACHE = {}
PROFILE = False  # test harness sets True to collect an NTFF trace


def _split_waits(nc, mybir):
    """TRN2 allows one sem wait per instruction (two for EventSemaphore);
    this walrus build rejects over-limit instructions, so move excess waits
    onto chained NOPs on the same engine just before the instruction."""
    ctr = 0
    for fn in nc.m.functions:
        for bb in fn.blocks:
            changed = False
            out = []
            for inst in bb.instructions:
                si = inst.sync_info
                cap = 2 if isinstance(inst, mybir.InstEventSemaphore) else 1
                if si is not None and si.on_wait and len(si.on_wait) > cap:
                    waits = list(si.on_wait)
                    for w in waits[:-cap]:
                        nop = mybir.InstNoOp(
                            name=f"wsplit-{ctr}", ins=[], outs=[]
                        )
                        ctr += 1
                        nop.engine = inst.engine
                        nop.sync_info = mybir.SyncInfo(on_wait=[w], on_update=[])
                        nc.register_instruction(nop)
                        out.append(nop)
                    si.on_wait = waits[-cap:]
                    changed = True
                out.append(inst)
            if changed:
                bb.instructions = out
    return ctr


def _build():
    import concourse.bass as bass
    import concourse.mybir as mybir
    import concourse.tile as tile

    dt = mybir.dt
    AF = mybir.ActivationFunctionType
    OP = mybir.AluOpType
    F32 = dt.float32
    F32R = dt.float32r
    F16 = dt.float16
    F8 = dt.float8e4

    nc = bass.Bass()

    # ---- DRAM I/O ----
    # per-group affine coefs in turns: pts_t = A + B * t
    a_d = nc.dram_tensor("acf", [NGRP, 128, 384], F32, kind="ExternalInput")
    b_d = nc.dram_tensor("bcf", [NGRP, 128, 384], F32, kind="ExternalInput")
    tq_d = nc.dram_tensor("tq", [NGRP, 128, 128], F32, kind="ExternalInput")
    w0_d = nc.dram_tensor("w0p", [40, 256], F16, kind="ExternalInput")
    if USE_FP8_L1:
        w1_d = nc.dram_tensor("w1dr", [128, 512], F8, kind="ExternalInput")
    else:
        w1_d = nc.dram_tensor("w1", [256, 256], F16, kind="ExternalInput")
    w2_d = nc.dram_tensor("w2h", [128, 8], F16, kind="ExternalInput")
    b1_d = nc.dram_tensor("b1t", [128, 2], F32, kind="ExternalInput")
    b2_d = nc.dram_tensor("b2t", [128, 4], F32, kind="ExternalInput")
    ltri_d = nc.dram_tensor("ltri", [128, 256], F32, kind="ExternalInput")
    sel2_d = nc.dram_tensor("sel2", [128, 2], F32R, kind="ExternalInput")
    identh_d = nc.dram_tensor("identh", [128, 128], F16, kind="ExternalInput")
    out_d = nc.dram_tensor("out", [NGRP, 2, 384], F32, kind="ExternalOutput")

    HB = 64  # blocks per half
    TPH = HB // 4  # 512-point tiles per half

    with tile.TileContext(nc) as tc:
        with (
            tc.tile_pool(name="consts", bufs=1) as cpool,
            tc.tile_pool(name="o2", bufs=NGRP) as o2pool,
        ):
            # ---- constants / weights (small, upfront) ----
            # w0 at partitions 0-39 and a replica at 64-103 (L0 row-packing)
            w0s = cpool.tile([128, 256], F16, tag="w0s")
            nc.sync.dma_start(w0s[0:40, :], w0_d[:])
            nc.sync.dma_start(w0s[64:104, :], w0_d[:])
            if USE_FP8_L1:
                w1dr = cpool.tile([128, 512], F8, tag="w1dr")
                nc.sync.dma_start(w1dr[:], w1_d[:])
                w1v = w1dr.rearrange("p (k m) -> p k m", k=2)
            else:
                w1s0 = cpool.tile([128, 256], F16, tag="w1s0")
                nc.sync.dma_start(w1s0[:], w1_d[0:128, :])
                w1s1 = cpool.tile([128, 256], F16, tag="w1s1")
                nc.sync.dma_start(w1s1[:], w1_d[128:256, :])
            w2s = cpool.tile([128, 8], F16, tag="w2s")
            nc.sync.dma_start(w2s[:], w2_d[:])
            b1t = cpool.tile([128, 2], F32, tag="b1t")
            nc.sync.dma_start(b1t[:], b1_d[:])
            b2t = cpool.tile([128, 4], F32, tag="b2t")
            nc.sync.dma_start(b2t[:], b2_d[:])
            ltri = cpool.tile([128, 256], F32, tag="ltri")
            nc.sync.dma_start(ltri[:], ltri_d[:])
            sel2 = cpool.tile([128, 2], F32R, tag="sel2")
            nc.sync.dma_start(sel2[:], sel2_d[:])
            identh = cpool.tile([128, 128], F16, tag="identh")
            nc.sync.dma_start(identh[:], identh_d[:])
            zerot = cpool.tile([128, 1], F32, tag="zerot")
            nc.vector.memset(zerot[:], 0.0)

            with (
                tc.tile_pool(name="gin", bufs=2) as ginpool,
                tc.tile_pool(name="gpt", bufs=2) as gptpool,
                tc.tile_pool(name="sa", bufs=2) as sapool,
                tc.tile_pool(name="kb", bufs=2) as kbpool,
                tc.tile_pool(name="sf", bufs=2) as sfpool,
                tc.tile_pool(name="fs", bufs=4) as fspool,
                tc.tile_pool(name="h0s", bufs=2) as h0spool,
                tc.tile_pool(name="h1s", bufs=4) as h1spool,
                tc.tile_pool(name="tpP", bufs=1, space="PSUM") as tp_pool,
                tc.tile_pool(name="h0P", bufs=2, space="PSUM") as h0_pool,
                tc.tile_pool(name="h1P", bufs=2, space="PSUM") as h1_pool,
                tc.tile_pool(name="oP", bufs=1, space="PSUM") as o_pool,
            ):
                o2s = []
                for g in range(NGRP):
                    # ---- phase A (per group): pts in turns + range reduce
                    # First groups' loads go on the second (Activation) DGE
                    # ring so the startup DMA burst runs 2-wide.
                    dge = nc.scalar if g < 2 else nc.sync
                    at = ginpool.tile([128, 384], F32, tag="at")
                    dge.dma_start(at[:], a_d[g])
                    bt = ginpool.tile([128, 384], F32, tag="bt")
                    dge.dma_start(bt[:], b_d[g])
                    tqg = ginpool.tile([128, 128], F32, tag="tqg")
                    dge.dma_start(tqg[:], tq_d[g])

                    ptst = gptpool.tile([128, 384], F32, tag="ptst")
                    ptsr = gptpool.tile([128, 384], F32, tag="ptsr")
                    praw = gptpool.tile([128, 384], F32, tag="praw")
                    tqb = tqg.unsqueeze(1).broadcast_to([128, 3, 128])
                    p3 = ptst.rearrange("p (c j) -> p c j", c=3)
                    nc.vector.tensor_tensor(p3, bt.rearrange("p (c j) -> p c j", c=3), tqb, op=OP.mult)
                    nc.vector.tensor_tensor(ptst[:], ptst[:], at[:], op=OP.add)
                    # raw radians for features 36-38
                    nc.vector.tensor_scalar(praw[:], ptst[:], TWO_PI, None, op0=OP.mult)
                    # r0 = ptst - round(ptst)  (exact in fp32)
                    kg = gptpool.tile([128, 384], F32, tag="kg")
                    nc.vector.tensor_scalar(kg[:], ptst[:], MAGIC, None, op0=OP.add)
                    nc.vector.tensor_scalar(kg[:], kg[:], MAGIC, None, op0=OP.subtract)
                    nc.vector.tensor_tensor(ptsr[:], ptst[:], kg[:], op=OP.subtract)

                    og = o_pool.tile([128, 512], F32, tag="og")

                    # Software pipeline: L1 for tile t is emitted during
                    # tile t+1 and L2 during t+2, so the in-order PE queue
                    # never waits on same-tile activation drains.
                    pend_l1 = []
                    pend_l2 = []

                    def emit_l2(h1pair, jbase):
                        for jp in range(4):
                            jj = jbase + jp
                            nc.tensor.matmul(
                                og[:, 4 * jj : 4 * (jj + 1)],
                                h1pair[1][:, 128 * jp : 128 * (jp + 1)],
                                w2s[:, 4:8],
                                start=True,
                                stop=False,
                            )
                            nc.tensor.matmul(
                                og[:, 4 * jj : 4 * (jj + 1)],
                                h1pair[0][:, 128 * jp : 128 * (jp + 1)],
                                w2s[:, 0:4],
                                start=False,
                                stop=True,
                            )

                    def emit_l1(h0s, jbase):
                        # half 1 first so its vector drain overlaps half 0's
                        # matmuls; L2 consumes half 1 (start) before half 0.
                        h1ss = [None, None]
                        for h in (1, 0):
                            h1p = h1_pool.tile([128, 512], F32, tag="h1p")
                            nc.tensor.matmul(
                                h1p[:],
                                w1s0[:, 128 * h : 128 * (h + 1)],
                                h0s[:, 0:512],
                                start=True,
                                stop=False,
                            )
                            nc.tensor.matmul(
                                h1p[:],
                                w1s1[:, 128 * h : 128 * (h + 1)],
                                h0s[:, 512:1024],
                                start=False,
                                stop=True,
                            )
                            h1s = h1spool.tile([128, 512], F16, tag="h1s")
                            with tc.high_priority():
                                if h == 1:
                                    nc.vector.tensor_scalar(
                                        h1s[:], h1p[:], b1t[:, 1:2], 0.0,
                                        op0=OP.add, op1=OP.max,
                                    )
                                else:
                                    nc.scalar.activation(
                                        h1s[:], h1p[:], AF.Relu,
                                        bias=b1t[:, 0:1],
                                    )
                            h1ss[h] = h1s
                        pend_l2.append((h1ss, jbase))

                    for half in range(2):
                        # ---- per-half: doubling chain (turns) + sin ----
                        sa = sapool.tile([128, 2304], F32, tag="sa")
                        sav = sa.rearrange("p (f j) -> p f j", j=HB)
                        # r0 half-view [128, 3, 64]
                        r0v = ptsr.rearrange("p (c j) -> p c j", c=3)[
                            :, :, HB * half : HB * (half + 1)
                        ]
                        prv = praw.rearrange("p (c j) -> p c j", c=3)[
                            :, :, HB * half : HB * (half + 1)
                        ]
                        # sin chunk l=0
                        nc.vector.tensor_copy(sav[:, 0:3, :], r0v)
                        # sin l=1..5: r_l = 2 r_{l-1} - round(2 r_{l-1})
                        for l in range(1, L):
                            prev = sa[:, 192 * (l - 1) : 192 * l]
                            cur = sa[:, 192 * l : 192 * (l + 1)]
                            kb = kbpool.tile([128, 192], F32, tag="kb")
                            nc.vector.tensor_scalar(
                                kb[:], prev, 2.0, MAGIC, op0=OP.mult, op1=OP.add
                            )
                            nc.vector.tensor_scalar(
                                kb[:], kb[:], MAGIC, None, op0=OP.subtract
                            )
                            nc.vector.scalar_tensor_tensor(
                                cur, prev, 2.0, kb[:], op0=OP.mult, op1=OP.subtract
                            )
                        # cos l: c_l = r_l + 0.25 - [r_l > 0.25]
                        for l in range(L):
                            rl = sa[:, 192 * l : 192 * (l + 1)]
                            cl = sa[:, 1152 + 192 * l : 1152 + 192 * (l + 1)]
                            kb = kbpool.tile([128, 192], F32, tag="kb")
                            nc.vector.tensor_scalar(
                                kb[:], rl, 0.25, 1.0, op0=OP.is_gt, op1=OP.mult
                            )
                            nc.vector.scalar_tensor_tensor(
                                cl, rl, 0.25, kb[:], op0=OP.add, op1=OP.subtract
                            )
                        # sf (f-major): 0:2304 trig, 2304:2496 raw, 2496:2560 ones
                        sf = sfpool.tile([128, 2560], F16, tag="sf")
                        nc.scalar.activation(
                            sf[:, 0:2304], sa[:], AF.Sin, scale=TWO_PI
                        )
                        nc.vector.tensor_copy(
                            sf.rearrange("p (f j) -> p f j", j=HB)[:, 36:39, :], prv
                        )
                        nc.vector.memset(sf[:, 2496:2560], 1.0)

                        sfv = sf.rearrange("p (f j) -> p f j", j=HB)
                        for itl in range(TPH):
                            # PE-transpose 4 blocks -> feature-major fp16.
                            # Two 1KB halves of one PSUM bank, manual ping-pong.
                            tpfull = tp_pool.tile([40, 1024], F16, tag="tp")
                            tp = tpfull[:, 512 * (itl % 2) : 512 * (itl % 2) + 512]
                            for jp in range(4):
                                jj = 4 * itl + jp
                                nc.tensor.transpose(
                                    tp[:, 128 * jp : 128 * (jp + 1)],
                                    sfv[:, :, jj],
                                    identh[:],
                                )
                            fs = fspool.tile([128, 512], F16, tag="fs")
                            with tc.high_priority():
                                nc.vector.tensor_copy(fs[0:40, :], tp[:])
                            # duplicate features at partitions 64-103 so L0
                            # can run as two concurrent row-tiles
                            nc.sync.dma_start(fs[64:104, :], fs[0:40, :])
                            # L0 (bias folded via ones row): one concurrent
                            # row-tile pair — rows 0-39 compute hidden half 0,
                            # rows 64-103 (duplicated features) compute half 1.
                            # Outputs land in different PSUM banks.
                            h0p = h0_pool.tile([128, 1024], F32, tag="h0p")
                            nc.tensor.matmul(
                                h0p[:, 0:512], w0s[0:40, 0:128], fs[0:40, :]
                            )
                            nc.tensor.matmul(
                                h0p[:, 512:1024], w0s[64:104, 128:256],
                                fs[64:104, :],
                            )
                            h0s = h0spool.tile([128, 1024], F16, tag="h0s")
                            with tc.high_priority():
                                nc.scalar.activation(h0s[:], h0p[:], AF.Relu)
                            # L1 inline; emit the PREVIOUS tile's L2 after it
                            emit_l1(h0s, 64 * half + 4 * itl)
                            if len(pend_l2) > 1:
                                emit_l2(*pend_l2.pop(0))
                    while pend_l2:
                        emit_l2(*pend_l2.pop(0))
                    # ---- drain O psum -> O2 sbuf (bias, relu on sigma) ----
                    o2 = o2pool.tile([128, 512], F32, tag="o2")
                    orgb = og.rearrange("p (j c) -> p j c", c=4)[:, :, 0:3]
                    o2rgb = o2.rearrange("p (j c) -> p j c", c=4)[:, :, 0:3]
                    brgb = b2t[:, 0:3].unsqueeze(1).broadcast_to([128, 128, 3])
                    nc.vector.tensor_tensor(o2rgb, orgb, brgb, op=OP.add)
                    osig = og.rearrange("p (j c) -> p j c", c=4)[:, :, 3]
                    o2sig = o2.rearrange("p (j c) -> p j c", c=4)[:, :, 3]
                    zbc = zerot[:, 0:1].broadcast_to([128, 128])
                    nc.vector.scalar_tensor_tensor(
                        o2sig, osig, b2t[:, 3:4], zbc, op0=OP.add, op1=OP.max
                    )
                    o2s.append(o2)

            # ---- phase C: compositing (tanh-sigmoid + exp scans) ----
            tc.no_sync_barrier()
            with (
                tc.tile_pool(name="cS", bufs=2) as cspool,
                tc.tile_pool(name="cP", bufs=2, space="PSUM") as cppool,
            ):
                for g in range(NGRP):
                    o2 = o2s[g]
                    o2v = o2.rearrange("p (j c) -> p j c", c=4)
                    # sigmoid(x) = 0.5*tanh(x/2) + 0.5 (0.5s folded into sel2)
                    e = cspool.tile([128, 384], F32, tag="e")
                    nc.scalar.activation(
                        e.rearrange("p (j c) -> p j c", c=3),
                        o2v[:, :, 0:3],
                        AF.Tanh,
                        scale=0.5,
                    )
                    # scans: exclusive & inclusive cumsum of sigma over s
                    scp = cppool.tile([128, 256], F32, tag="scp")
                    sig = o2v[:, :, 3]
                    nc.tensor.matmul(scp[:, 0:128], ltri[:, 0:128], sig)
                    nc.tensor.matmul(scp[:, 128:256], ltri[:, 128:256], sig)
                    texin = cspool.tile([128, 256], F32, tag="texin")
                    nc.scalar.activation(texin[:], scp[:], AF.Exp, scale=-DELTA)
                    wt = cspool.tile([128, 128], F32, tag="wt")
                    nc.vector.tensor_tensor(
                        wt[:], texin[:, 0:128], texin[:, 128:256], op=OP.subtract
                    )
                    # wr = (tanh + 1) * w   (the 0.5 lives in sel2)
                    wr = cspool.tile([128, 384], F32R, tag="wr")
                    nc.vector.scalar_tensor_tensor(
                        wr.rearrange("p (j c) -> p j c", c=3),
                        e.rearrange("p (j c) -> p j c", c=3),
                        1.0,
                        wt.unsqueeze(2).broadcast_to([128, 128, 3]),
                        op0=OP.add,
                        op1=OP.mult,
                    )
                    rp_ = cppool.tile([2, 384], F32, tag="rp")
                    nc.tensor.matmul(rp_[:], sel2[:], wr[:])
                    outs = cspool.tile([2, 384], F32, tag="outs")
                    nc.vector.tensor_copy(outs[:], rp_[:])
                    nc.sync.dma_start(out_d[g], outs[:])

    _split_waits(nc, mybir)
    return nc


def _host_prep(origins, directions, t_rand, W0, b0, W1, b1, W2, b2):
    """Build per-core input maps (all numpy, cheap)."""
    import ml_dtypes

    f32 = np.float32
    # F-row order: rows 3l+c = sin freq l coord c; 18+3l+c = cos; 36..38 pts;
    # row 39 = bias (ones feature).
    perm = np.zeros(39, np.int64)
    perm[36:39] = (0, 1, 2)
    for l in range(L):
        for c in range(3):
            perm[3 * l + c] = 3 + 6 * l + c
            perm[18 + 3 * l + c] = 3 + 6 * l + 3 + c
    w0p = np.empty((40, 256), np.float16)
    w0p[0:39] = W0[perm].astype(np.float16)
    w0p[39] = b0.astype(np.float16)

    if USE_FP8_L1:
        # [p, k*256 + m] = W1[k*128 + p, m]
        w1dr = np.empty((128, 512), ml_dtypes.float8_e4m3)
        w1dr[:, 0:256] = W1[0:128, :].astype(ml_dtypes.float8_e4m3)
        w1dr[:, 256:512] = W1[128:256, :].astype(ml_dtypes.float8_e4m3)
        w1_payload = ("w1dr", w1dr)
    else:
        w1_payload = ("w1", W1.astype(np.float16))

    w2h = np.empty((128, 8), np.float16)
    w2h[:, 0:4] = W2[0:128].astype(np.float16)
    w2h[:, 4:8] = W2[128:256].astype(np.float16)
    b1t = np.ascontiguousarray(b1.reshape(2, 128).T).astype(f32)
    b2t = np.broadcast_to(b2.astype(f32), (128, 4)).copy()

    q = np.arange(128)
    rp = q // 64
    s = q % 64
    zc = (NEAR + DELTA * s).astype(np.float64)  # [128] per-partition z offset

    # ltri: cols 0..127 exclusive, 128..255 inclusive
    kk = q
    krp = kk // 64
    kj = kk % 64
    same = (krp[:, None] == rp[None, :])
    ltri = np.zeros((128, 256), f32)
    ltri[:, 0:128] = (same & (kj[:, None] < s[None, :])).astype(f32)
    ltri[:, 128:256] = (same & (kj[:, None] <= s[None, :])).astype(f32)
    # 0.5 from the tanh-sigmoid identity folded in here
    sel2 = (0.5 * (krp[:, None] == np.arange(2)[None, :])).astype(f32)
    identh = np.eye(128, dtype=np.float16)

    # ray_of[J, rp] = 16*(J%128) + 2*(J//128) + rp
    J = np.arange(NBLK)
    ray_of = (16 * (J % 128))[:, None] + (2 * (J // 128))[:, None] + np.arange(2)[None, :]

    in_maps = []
    for core in range(NCORES):
        o = origins[core * BC : (core + 1) * BC].astype(np.float64)
        d = directions[core * BC : (core + 1) * BC].astype(np.float64)
        t = t_rand[core * BC : (core + 1) * BC].astype(f32)
        rays_qJ = ray_of[:, :].T[rp]  # [128, NBLK]: ray_of[J, rp(q)]
        # tq[q, J] = t[ray, s(q)]
        tq = t[rays_qJ, s[:, None]]  # [128, NBLK]
        tq = np.ascontiguousarray(tq.reshape(128, NGRP, 128).transpose(1, 0, 2))
        # affine coefs in turns: pts_t[c] = A + B * t
        # x_c = o_c + d_c * (zc(q) + DELTA * t)
        oe = o[rays_qJ]  # [128, NBLK, 3]
        de = d[rays_qJ]
        A = (oe + de * zc[:, None, None]) * INV2PI   # [128, NBLK, 3]
        Bc = de * (DELTA * INV2PI)
        # -> [NGRP, 128, 384] with col = c*128 + j
        A = A.reshape(128, NGRP, 128, 3).transpose(1, 0, 3, 2).reshape(NGRP, 128, 384)
        Bc = Bc.reshape(128, NGRP, 128, 3).transpose(1, 0, 3, 2).reshape(NGRP, 128, 384)
        in_map = {
            "acf": np.ascontiguousarray(A).astype(f32),
            "bcf": np.ascontiguousarray(Bc).astype(f32),
            "tq": tq,
            "w0p": w0p,
            w1_payload[0]: w1_payload[1],
            "w2h": w2h,
            "b1t": b1t,
            "b2t": b2t,
            "ltri": ltri,
            "sel2": sel2,
            "identh": identh,
        }
        in_maps.append(in_map)
    return in_maps


def kernel(origins, directions, t_rand, W0, b0, W1, b1, W2, b2, near, far,
           **kw):
    assert int(near) == 2 and int(far) == 6
    from concourse.bass_utils import run_bass_kernel_spmd

    if "nc" not in _CACHE:
        _CACHE["nc"] = _build()
    nc = _CACHE["nc"]

    in_maps = _host_prep(
        np.asarray(origins), np.asarray(directions), np.asarray(t_rand),
        np.asarray(W0), np.asarray(b0), np.asarray(W1), np.asarray(b1),
        np.asarray(W2), np.asarray(b2),
    )
    res = run_bass_kernel_spmd(
        nc, in_maps, core_ids=list(range(NCORES)), trace=PROFILE
    )
    _CACHE["last_results"] = res
    out = np.empty((B, 3), np.float32)
    for core in range(NCORES):
        oc = res.results[core]["out"].reshape(NGRP, 2, 128, 3)
        # group g holds blocks J = 128*g + i ; ray = 16*i + 2*g + rp
        for g in range(NGRP):
            for rpp in range(2):
                rays = core * BC + 16 * np.arange(128) + 2 * g + rpp
                out[rays] = oc[g, rpp]
    return out
